# revision 53
# baseline (speedup 1.0000x reference)
"""Trainium2 Bass kernel for nn_CA_85332410237583.

Computation (B=8, C=8, H=W=256, F=4):
  k = totalistic(kernels)                       # D4-symmetrized 5x5, zero mean
  z = floor(x*PV2); p = floor(conv_circ(z, k) + bias)/PV2
  h = p; 4x [h = tanh(floor(W@floor(h*PV1))/PV1)]   (per-filter 1->32->32->32->8 MLP)
  z3 = sort(h, filters)[-3]; out = clip(x + z3*update_rate, 0, 1)

Key insight: the transition MLP input is a SCALAR per (filter, pixel) (the
1x1 conv stack has channel dim 1 at its input), so the entire 4-layer MLP +
tanh chain collapses into 32 scalar functions g_{f,c}: R -> R.  These are
tabulated host-side on an 8192-bin grid over the per-filter conv output
range and evaluated on-chip with gpsimd ap_gather.  This removes ~620M MACs
and ~25M tanh evaluations per image.

Per-core pipeline (one image per NeuronCore, batch-parallel over 8 cores):
  * conv: 25 taps x2 fp32r matmuls per 1024-px column tile, K=128=(blk,c),
    M=64=(f,blk), N=512; a tiny-matmul warm-up holds the PE at full clock.
  * ACT: idx = round(p*scale_f + c0_f) cast to int16, written (px%16)-grouped
    so the gather's wrapped index layout is reachable with 512B DMA runs.
  * idx bounce: SBUF -> DRAM -> ap_gather's interleaved [128, S/16] wrap.
  * gather: ap_gather per half-image (2 sub-gathers), groups=(f,h),
    16 partitions/group = 8 channels x2, tables = g_{f,c} at bin centers
    (final tanh and update_rate baked in).
  * shuffle: 32 DMAs/half move u_{f,c} planes into the x-aligned layout
    [(blk,c), (f, px)] (contiguous 8KB runs).
  * sort: 7-op min/max network on DVE gives the 2nd-smallest across filters
    (mirrored if update_rate<0), then clip(x + z3) reading x from the f32
    halo frame; the result lands in dead planes of the shuffle buffer.
"""

import os
import numpy as np

import concourse.bass as bass
import concourse.bacc as bacc
import concourse.mybir as mybir
from concourse.tile import TileContext
from concourse.bass_utils import run_bass_kernel_spmd

F32 = mybir.dt.float32
F32R = mybir.dt.float32r
I16 = mybir.dt.int16
AF = mybir.ActivationFunctionType
ALU = mybir.AluOpType

B, C, H, W = 8, 8, 256, 256
F = 4
RK, HALO = 5, 2
PV1 = float(np.floor(2**31 / 128))
PV2 = float(np.floor(2**31 / (RK * RK * 128)))

NBLK, RB = 16, 16          # 16 row-blocks of 16 rows
ROWS, COLS = RB + 2 * HALO, W + 2 * HALO      # 20, 260
FREE = ROWS * COLS                            # 5200 per partition
NPIX = RB * W                                 # 4096 pixels per block
CT = 4                                        # column tiles of 1024
CTW = NPIX // CT                              # 1024
NB = 8192                  # table bins per g_{f,c}
HPX = NPIX // 2            # 2048 pixels per block per half
NIDX = 16384               # gather stream length per half (8 blocks x 2048)

_cache = {}
LAST_RESULTS = None


def _totalistic(k):
    def sym(a):
        return a + np.flip(a, -2) + np.flip(a, -1) + np.flip(a, (-2, -1))
    z = 0.125 * (sym(k) + sym(np.swapaxes(k, -2, -1)))
    return z - z.mean(axis=(-2, -1), keepdims=True)


def _prep_weights(kernels, biases, W1, W2, W3, W4, ur):
    kt64 = _totalistic(kernels.astype(np.float64))
    kt = kt64.astype(np.float32)                  # [F,C,5,5]
    beff = biases.astype(np.float64) / PV2

    # conv lhsT: [128=(blk,c), 25*64]; col tap*64 + (f*16+blk)
    convw = np.zeros((128, 25 * 64), np.float32)
    for t in range(25):
        dy, dx = divmod(t, 5)
        for blk in range(NBLK):
            for c in range(C):
                for f in range(F):
                    convw[blk * 8 + c, t * 64 + f * 16 + blk] = kt[f, c, dy, dx]

    # per-filter conv-output range (x in [0,1)), small margin for fp32r
    # rounding inside the PE accumulation.
    ktf = kt.astype(np.float64)
    qmax = np.maximum(ktf, 0).sum(axis=(1, 2, 3))
    qmin = np.minimum(ktf, 0).sum(axis=(1, 2, 3))
    rng = qmax - qmin
    qmin -= 0.005 * rng + 1e-3
    qmax += 0.005 * rng + 1e-3
    dp = (qmax - qmin) / (NB - 1)
    scale = 1.0 / dp
    c0 = -qmin * scale

    # tables: g_{f,c} on the bin-center grid, with the reference's fixed-point
    # quantization, the final tanh, and update_rate baked in.
    Ws = (W1.astype(np.float64), W2.astype(np.float64),
          W3.astype(np.float64), W4.astype(np.float64))
    tbl = np.zeros((128, NB), np.float32)
    for f in range(F):
        q = qmin[f] + np.arange(NB, dtype=np.float64) * dp[f]
        h = (q + beff[f])[None, :]                # [1, NB]
        for Wm in Ws:
            h = np.floor(h * PV1)
            h = Wm[f] @ h
            h = np.tanh(np.floor(h) / PV1)
        val = (h * ur).astype(np.float32)         # [8, NB]
        for hh in range(2):
            for k in range(16):
                tbl[f * 32 + hh * 16 + k] = val[k % 8]

    scl = np.zeros((64, 1), np.float32)
    bia = np.zeros((64, 1), np.float32)
    for f in range(F):
        scl[f * 16:(f + 1) * 16, 0] = scale[f]
        bia[f * 16:(f + 1) * 16, 0] = c0[f]
    return convw, tbl, scl, bia


def _stage_x(xb):
    """xb: [C, H, W] -> f32 [128=(blk,c), ROWS*COLS] with circular halo."""
    halo = np.empty((128, ROWS, COLS), np.float32)
    rows = (np.arange(-HALO, RB + HALO)[None, :] + np.arange(NBLK)[:, None] * RB) % H
    cols = np.arange(-HALO, W + HALO) % W
    for blk in range(NBLK):
        halo[blk * 8:blk * 8 + 8] = xb[:, rows[blk]][:, :, cols]
    return halo.reshape(128, FREE)


def _build_nc(ur):
    nc = bacc.Bacc(trn_type="TRN2")

    xd = nc.dram_tensor("xsb", [128, FREE], F32R, kind="ExternalInput")
    cwd = nc.dram_tensor("convw", [128, 1600], F32R, kind="ExternalInput")
    tbld = nc.dram_tensor("tbl", [128, NB], F32, kind="ExternalInput")
    scld = nc.dram_tensor("scl", [64, 1], F32, kind="ExternalInput")
    biad = nc.dram_tensor("bia", [64, 1], F32, kind="ExternalInput")
    outd = nc.dram_tensor("out", [128, NPIX], F32, kind="ExternalOutput")
    fidxd = nc.dram_tensor("fidxd", [64, NPIX], I16, kind="Internal")

    second_smallest = ur >= 0

    with TileContext(nc) as tc:
        with (
            tc.tile_pool(name="w", bufs=1) as wp,
            tc.tile_pool(name="uh", bufs=1) as up,
            tc.tile_pool(name="u2", bufs=1) as u2p,
            tc.tile_pool(name="st", bufs=1) as stp,
            tc.tile_pool(name="ps", bufs=2, space="PSUM") as pp,
        ):
            xw = wp.tile([128, FREE], F32R, tag="xw")
            cw = wp.tile([128, 1600], F32R, tag="cw")
            tbl = wp.tile([128, NB], F32, tag="tbl")
            scl = wp.tile([64, 1], F32, tag="scl")
            bia = wp.tile([64, 1], F32, tag="bia")
            fidx2 = wp.tile([64, HPX], I16, tag="fidx2")
            idxw = wp.tile([128, 2048], I16, tag="idxw")

            nc.sync.dma_start(out=scl[:], in_=scld[:])
            nc.sync.dma_start(out=bia[:], in_=biad[:])
            nc.sync.dma_start(out=cw[:], in_=cwd[:])
            # x halo frame in two chunks: conv ct0 only needs frame rows 0-8,
            # so the first matmul can start ~2us in while the rest streams.
            XC1 = 9 * COLS
            nc.sync.dma_start(out=xw[:, 0:XC1], in_=xd[:, 0:XC1])
            nc.sync.dma_start(out=xw[:, XC1:FREE], in_=xd[:, XC1:FREE])

            # PE pstate warm-up: tiny matmuls on a zeroed tile run during the
            # convw/x DMAs so the conv starts at full clock with no PE gap.
            wz = wp.tile([128, 256], F32R, tag="wz")
            nc.vector.memset(wz[:].bitcast(F32), 0.0)
            wps = pp.tile([128, 64], F32, tag="warm", bufs=1)
            for i in range(64):
                nc.tensor.matmul(wps[:, :], lhsT=wz[:, 0:128], rhs=wz[:, 0:64],
                                 start=(i == 0), stop=(i == 63))

            xr = xw[:].rearrange("p (r c) -> p r c", c=COLS)   # [128, 20, 260]
            # fidx2 (one half live at a time): slot = k*128 + tg,
            # where px_in_half = tg*16 + k, tg = ct2*64 + t'.
            fidx_v = fidx2[:].rearrange("p (k t) -> p k t", k=16)
            # idx bounce source view: dims (f, h, hh, k, bl, tg)
            fidxd_v = fidxd[:].rearrange(
                "(f h bl) (hh k t) -> f h hh k bl t", f=4, h=2, hh=2, k=16)

            def conv_and_idx(ct):
                # ---- conv: 25 taps accumulate into p psum [64, 1024] ----
                pps = pp.tile([64, CTW], F32, tag="acc", bufs=3,
                              name=f"pps_{ct}")
                for t in range(25):
                    dy, dx = divmod(t, 5)
                    for s in range(2):
                        r0 = 4 * ct + 2 * s + dy
                        rhs = xr[:, r0:r0 + 2, dx:dx + W]
                        outap = pps[0:64, s * 512:(s + 1) * 512].rearrange(
                            "p (a b) -> p a b", b=W)
                        nc.tensor.matmul(
                            outap,
                            lhsT=cw[:, t * 64:t * 64 + 64],
                            rhs=rhs,
                            start=(t == 0), stop=(t == 24),
                        )
                # ---- idx = round(p*scale + c0) as int16, (k,t)-grouped ----
                ct2 = ct % 2
                inv = pps[0:64, :].rearrange("p (t k) -> p k t", k=16)
                outv = fidx_v[:, :, ct2 * 64:(ct2 + 1) * 64]
                nc.scalar.activation(outv, inv, AF.Identity,
                                     bias=bia[:], scale=scl[:])

            def idx_bounce(half):
                # SBUF -> DRAM -> wrapped idx layout (ACT queue, after acts)
                hs = slice(half * HPX, (half + 1) * HPX)
                nc.scalar.dma_start(out=fidxd[:, hs], in_=fidx2[:])
                for f in range(F):
                    for h in range(2):
                        nc.scalar.dma_start(
                            out=idxw[f * 32 + h * 16:f * 32 + h * 16 + 16,
                                     half * 1024:(half + 1) * 1024],
                            in_=fidxd_v[f, h, half])

            def gather(half, u_h):
                # u[(f,h,k), j] = g_{f,k%8}(p at pixel j); 2 sub-gathers
                for q in range(2):
                    nc.gpsimd.ap_gather(
                        out_ap=u_h[:, q * (NIDX // 2):(q + 1) * (NIDX // 2)],
                        in_ap=tbl[:],
                        idxs_ap=idxw[:, half * 1024 + q * 512:
                                     half * 1024 + (q + 1) * 512],
                        channels=128,
                        num_elems=NB,
                        d=1,
                        num_idxs=NIDX // 2,
                    )

            def shuffle(half, u_h, U2, eng):
                # to x-aligned planes: U2[(blk,c), f*HPX+pxh]
                for f in range(F):
                    for c in range(C):
                        eng.dma_start(
                            out=U2[c:128:8, f * HPX:(f + 1) * HPX],
                            in_=u_h[f * 32 + c:f * 32 + 32:16, :])

            def sort_and_out(half, U2):
                # ---- 2nd-smallest (ur>=0) / 2nd-largest (ur<0) of 4 ----
                # 512-wide sub-chunks on DVE; P1 doubles as a temp once
                # consumed; x comes from the f32 halo frame; the final sum
                # lands in U2's dead P2 plane.
                lo, hi = (ALU.min, ALU.max) if second_smallest else (ALU.max, ALU.min)
                for sc in range(4):
                    t1 = stp.tile([128, 512], F32, tag=f"t1{sc % 2}",
                                  name=f"t1_{half}_{sc}")
                    P = [U2[:, f * HPX + sc * 512:f * HPX + (sc + 1) * 512]
                         for f in range(F)]
                    nc.vector.tensor_tensor(t1[:], P[0], P[1], lo)
                    nc.vector.tensor_tensor(P[0], P[0], P[1], hi)
                    nc.vector.tensor_tensor(P[1], P[2], P[3], lo)
                    nc.vector.tensor_tensor(P[2], P[2], P[3], hi)
                    nc.vector.tensor_tensor(t1[:], t1[:], P[1], hi)
                    nc.vector.tensor_tensor(P[0], P[0], P[2], lo)
                    nc.vector.tensor_tensor(t1[:], t1[:], P[0], lo)
                    # ---- out = clip(x + z3, 0, 1) into U2's P2 plane ----
                    r0 = HALO + half * 8 + sc * 2
                    xvc = xr[:, r0:r0 + 2, HALO:HALO + W].bitcast(F32)
                    osb = U2[:, 2 * HPX + sc * 512:2 * HPX + (sc + 1) * 512]
                    ov = osb.rearrange("p (a b) -> p a b", b=W)
                    tv = t1[:].rearrange("p (a b) -> p a b", b=W)
                    nc.vector.tensor_tensor(ov, xvc, tv, ALU.add)
                    nc.vector.tensor_scalar(osb, osb, 0.0, 1.0,
                                            ALU.max, ALU.min)
                nc.scalar.dma_start(out=outd[:, half * HPX:(half + 1) * HPX],
                                    in_=U2[:, 2 * HPX:3 * HPX])

            u_hs = [up.tile([128, NIDX], F32, tag="uh", bufs=1,
                            name=f"uh_{h}") for h in range(2)]
            U2s = [u2p.tile([128, 4 * HPX], F32, tag="u2", bufs=2,
                            name=f"u2_{h}") for h in range(2)]

            conv_and_idx(0)
            # table DMA emitted after the first conv => lower scheduler
            # priority than x/convw, so it doesn't delay the conv start.
            nc.sync.dma_start(out=tbl[:], in_=tbld[:])
            conv_and_idx(1)
            idx_bounce(0)
            gather(0, u_hs[0])
            conv_and_idx(2)
            shuffle(0, u_hs[0], U2s[0], nc.sync)
            conv_and_idx(3)
            idx_bounce(1)
            gather(1, u_hs[1])
            sort_and_out(0, U2s[0])
            shuffle(1, u_hs[1], U2s[1], nc.scalar)
            sort_and_out(1, U2s[1])
    nc.finalize()
    return nc


def kernel(x, kernels, biases, W1, W2, W3, W4, update_rate):
    global LAST_RESULTS
    x = np.ascontiguousarray(np.asarray(x, dtype=np.float32))
    kernels = np.asarray(kernels, dtype=np.float32)
    biases = np.asarray(biases, dtype=np.float32)
    W1 = np.asarray(W1, dtype=np.float32)
    W2 = np.asarray(W2, dtype=np.float32)
    W3 = np.asarray(W3, dtype=np.float32)
    W4 = np.asarray(W4, dtype=np.float32)
    ur = float(np.asarray(update_rate))

    key = ("nc", ur >= 0)
    if key not in _cache:
        _cache[key] = _build_nc(ur)
    nc = _cache[key]

    convw, tbl, scl, bia = _prep_weights(kernels, biases, W1, W2, W3, W4, ur)
    shared = {
        "convw": np.ascontiguousarray(convw),
        "tbl": np.ascontiguousarray(tbl),
        "scl": np.ascontiguousarray(scl),
        "bia": np.ascontiguousarray(bia),
    }
    in_maps = []
    for b in range(B):
        m = dict(shared)
        m["xsb"] = np.ascontiguousarray(_stage_x(x[b]))
        in_maps.append(m)

    trace = bool(int(os.environ.get("KERNEL_TRACE", "0")))
    res = run_bass_kernel_spmd(nc, in_maps, list(range(B)), trace=trace)
    LAST_RESULTS = res

    out = np.empty((B, C, H, W), np.float32)
    for b in range(B):
        ob = res.results[b]["out"].reshape(NBLK, C, RB, W)
        out[b] = ob.transpose(1, 0, 2, 3).reshape(C, H, W)
    return out
